# revision 29
# baseline (speedup 1.0000x reference)
"""Trainium2 Bass kernel for AlignedQuestionEmbeddingLayer.

Computation (per batch element):
    C = relu(Xc @ W.T + b)            # [4096, 128]
    Q = relu(Xq @ W.T + b)            # [512, 128]
    S = C @ Q.T  (+ mask)             # [4096, 512]
    A = softmax(S, axis=-1)
    out = A @ Q                       # [4096, 128]

Sharding: data-parallel over batch, one batch element per NeuronCore (8 cores).

Device-side design notes:
  - The dense layer contracts over E=300 (padded to 384 on host), so both
    matmul operands need E on the partition axis. Inputs are pre-transposed
    and fp16-cast on host: no on-device transposes of the big tensor, half
    the input DMA traffic, and fp16 matmuls run 4x faster than fp32 on the
    TRN2 PE (1 cycle/row vs 4).
  - The bias rides the E-padding: row 300 of xT is all-ones and row 300 of
    the W blob holds b, so both denses produce x@W.T+b straight out of the
    matmul and relu is a single DVE max (fp16/bf16 cast included).
  - One input DMA and one output DMA per 512-row super-tile (the Sync
    sequencer's serial DMA-trigger cost dominates otherwise).
  - Scores are computed transposed ([q partitions, c free]) so the final
    matmul (lhsT=expS_T chunk, rhs=[Q|ones]) produces both the output rows
    and the softmax denominators in a single PSUM accumulation; exp runs as
    two 1024-wide activations with bias=0 when the mask is all ones, and
    falls back to 4 per-chunk activations with the mask folded into the
    per-partition exp bias otherwise.
  - Softmax skips max-subtraction: scores are bounded (~|s|<40) so exp() is
    safe in fp32/bf16.
  - The loop is software-pipelined with lag 1 (final/norm/store of st-1
    between dense and scores of st) so the PE never waits on exp.
  - A burst of dummy matmuls at kernel start warms the PE HAM clock gate
    during the setup-DMA window, so real matmuls run at 2.4 GHz not 1.2.
"""

import sys

import numpy as np

sys.path.insert(0, "/opt/trn_rl_repo")

B, CTX, QST, E, H = 8, 4096, 512, 300, 128
N_CORES = 8
EP = 384            # E padded to 3 chunks of 128; row E carries the bias
ST = 512            # context rows per super-tile
N_ST = CTX // ST    # 8 super-tiles
N_WARM = 7          # dummy matmuls to warm the HAM clock gate

# fp16 blobs: W chunks (with bias row) and XqT chunks (with ones row)
BW_F = EP                 # bw: [:, k*128 : +128], k=0..2
BQ_F = 3 * 512            # bq: [:, k*512 : +512], k=0..2
# fp32 blob: bias column + 4 maskbias columns (only read by masked variant)
OFF_B = 0
OFF_MB = 1
BLOB32_F = 5

_COMPILED = {}


def _build_kernel(n_st=N_ST, masked=False):
    import concourse.bass as bass
    import concourse.tile as tile
    from concourse import bacc, mybir

    f32 = mybir.dt.float32
    f16 = mybir.dt.float16
    bf16 = mybir.dt.bfloat16
    AF = mybir.ActivationFunctionType
    MAX = mybir.AluOpType.max

    nc = bacc.Bacc(
        "TRN2", target_bir_lowering=False, debug=False, num_devices=N_CORES
    )

    xc4 = nc.declare_dram_parameter("xc4", [N_ST, 128, 3, ST], f16, isOutput=False)
    bw_d = nc.declare_dram_parameter("bw", [128, BW_F], f16, isOutput=False)
    bq_d = nc.declare_dram_parameter("bq", [128, BQ_F], f16, isOutput=False)
    b32_d = nc.declare_dram_parameter("b32", [128, BLOB32_F], f32, isOutput=False)
    out_d = nc.declare_dram_parameter("out4", [N_ST, 128, ST // 128, H], f32,
                                      isOutput=True)

    with tile.TileContext(nc) as tc:
        with (
            tc.tile_pool(name="const", bufs=1) as const_pool,
            tc.tile_pool(name="xin", bufs=4) as xin_pool,
            tc.tile_pool(name="ct", bufs=2) as ct_pool,
            tc.tile_pool(name="exps", bufs=2) as exps_pool,
            tc.tile_pool(name="outs", bufs=3) as outs_pool,
            tc.tile_pool(name="pct", bufs=1, space=bass.MemorySpace.PSUM) as pct_pool,
            tc.tile_pool(name="pst", bufs=1, space=bass.MemorySpace.PSUM) as pst_pool,
            tc.tile_pool(name="po", bufs=3, space=bass.MemorySpace.PSUM) as po_pool,
        ):
            # ---- PE warmup: matmuls on an uninitialized tile, results
            # discarded (next dense's start=True clears the bank) ----------
            warm = const_pool.tile([128, ST], f16, tag="warm")
            nc.gpsimd.memset(warm[:], 0.0)
            warm_ps = pct_pool.tile([H, ST], f32, tag="pct")
            for _ in range(N_WARM):
                nc.tensor.matmul(
                    warm_ps[:], warm[:, 0:128], warm[:], start=True, stop=True,
                    skip_group_check=True,
                )

            # ---- one-time setup -------------------------------------------
            bw_sb = const_pool.tile([128, BW_F], f16, tag="bw")
            nc.sync.dma_start(bw_sb[:], bw_d[:])
            bq_sb = const_pool.tile([128, BQ_F], f16, tag="bq")

            def load_bq():
                nc.sync.dma_start(bq_sb[:], bq_d[:])

            if masked:
                setup32 = const_pool.tile([128, BLOB32_F], f32, tag="setup32")
                nc.sync.dma_start(setup32[:], b32_d[:])

            def w_chunk(k):  # [128, 128] fp16 W.T chunk (row E holds b)
                return bw_sb[:, k * 128 : (k + 1) * 128]

            def q_chunk(k):  # [128, 512] fp16 Xq.T chunk (row E all-ones)
                return bq_sb[:, k * 512 : (k + 1) * 512]

            # question dense twice (bias rides the aug row):
            #   QT [h, q] fp16 for the scores lhsT
            #   Q  [q, h] bf16 (as [Q|ones] per chunk) for the final rhs
            qt_sb = const_pool.tile([H, QST], f16, tag="qt")
            qa_sb = []
            for j in range(4):
                qa_tile = const_pool.tile([128, H + 1], bf16, tag=f"qa{j}")
                qa_sb.append(qa_tile)

            def qt_setup():
                psum_q = pct_pool.tile([H, QST], f32, tag="pct")
                for k in range(3):
                    nc.tensor.matmul(
                        psum_q[:], w_chunk(k), q_chunk(k),
                        start=(k == 0), stop=(k == 2),
                    )
                nc.vector.tensor_scalar(qt_sb[:], psum_q[:], 0.0, None, MAX)

            def qa_setup():
                psum_qd = pst_pool.tile([128, 4 * ST], f32, tag="pst")
                for j in range(4):
                    for k in range(3):
                        nc.tensor.matmul(
                            psum_qd[:, j * 128 : (j + 1) * 128],
                            q_chunk(k)[:, j * 128 : (j + 1) * 128], w_chunk(k),
                            start=(k == 0), stop=(k == 2),
                        )
                for j in range(4):
                    nc.vector.tensor_scalar(
                        qa_sb[j][:, 0:H], psum_qd[:, j * 128 : (j + 1) * 128],
                        0.0, None, MAX,
                    )
                    nc.gpsimd.memset(qa_sb[j][:, H : H + 1], 1.0)

            # ---- software-pipelined main loop -----------------------------
            def load_phase(st, split=False):
                xa = xin_pool.tile([128, 3, ST], f16, tag="xa")
                if split:  # per-chunk DMAs so dense k=0 can start earliest
                    for k in range(3):
                        nc.sync.dma_start(xa[:, k, :], xc4[st, :, k, :])
                else:
                    nc.sync.dma_start(xa[:], xc4[st])
                return xa

            def dense_phase(xa):
                psum_ct = pct_pool.tile([H, ST], f32, tag="pct")
                for k in range(3):
                    nc.tensor.matmul(
                        psum_ct[:], w_chunk(k), xa[:, k, :],
                        start=(k == 0), stop=(k == 2),
                    )
                ct_sb = ct_pool.tile([H, ST], f16, tag="ct")
                nc.vector.tensor_scalar(ct_sb[:], psum_ct[:], 0.0, None, MAX)
                return ct_sb

            def scores_exp_phase(ct_sb):
                ps = pst_pool.tile([128, 4 * ST], f32, tag="pst")
                for j in range(4):
                    nc.tensor.matmul(
                        ps[:, j * ST : (j + 1) * ST],
                        qt_sb[:, j * 128 : (j + 1) * 128], ct_sb[:],
                        start=True, stop=True,
                    )
                es = exps_pool.tile([128, 4 * ST], bf16, tag="es")
                if masked:
                    for j in range(4):
                        nc.scalar.activation(
                            es[:, j * ST : (j + 1) * ST],
                            ps[:, j * ST : (j + 1) * ST], AF.Exp,
                            bias=setup32[:, OFF_MB + j : OFF_MB + j + 1],
                        )
                else:
                    nc.scalar.activation(es[:], ps[:], AF.Exp)
                return es

            def back_phase(st, es):
                def exp_chunk(ci, j):  # [128 q, 128 c] bf16 lhsT
                    return es[:, j * ST + ci * 128 : j * ST + (ci + 1) * 128]

                o_big = outs_pool.tile([128, ST // 128, H], f32, tag="obig")
                for cp in range(ST // 256):  # ci pairs share one PSUM bank
                    po2 = po_pool.tile([128, 2, H + 1], f32, tag="po")
                    for ch in range(2):
                        ci = 2 * cp + ch
                        for j in range(4):
                            nc.tensor.matmul(
                                po2[:, ch, :], exp_chunk(ci, j), qa_sb[j][:],
                                start=(j == 0), stop=(j == 3),
                            )
                    recip2 = outs_pool.tile([128, 2], f32, tag="recip")
                    sums2 = po2[:, :, H : H + 1].rearrange("p a b -> p (a b)")
                    nc.vector.reciprocal(recip2[:], sums2)
                    for ch in range(2):
                        ci = 2 * cp + ch
                        nc.vector.tensor_scalar_mul(
                            o_big[:, ci, :], po2[:, ch, 0:H],
                            recip2[:, ch : ch + 1])
                nc.sync.dma_start(out_d[st], o_big[:])

            # prologue: kick off the first three x loads, QT dense while
            # they land, dense+scores of st 0 as soon as x0 arrives, and
            # defer the (final-only) Q/qa dense past the first scores
            xas = {0: load_phase(0, split=True)}
            load_bq()
            for _st in range(1, min(3, n_st)):
                xas[_st] = load_phase(_st)
            cts = {0: dense_phase(xas.pop(0))}
            qt_setup()
            qa_setup()
            if n_st > 1:
                cts[1] = dense_phase(xas.pop(1))
            prev_exp = scores_exp_phase(cts[0])
            for st in range(1, n_st + 1):
                if st < n_st:
                    if st + 2 < n_st:
                        xas[st + 2] = load_phase(st + 2)
                    if st + 1 < n_st:
                        cts[st + 1] = dense_phase(xas.pop(st + 1))
                    back_phase(st - 1, prev_exp)
                    prev_exp = scores_exp_phase(cts.pop(st))
                else:
                    back_phase(st - 1, prev_exp)

    return nc


def _get_nc(masked=False):
    key = ("nc", masked)
    if key not in _COMPILED:
        nc = _build_kernel(masked=masked)
        nc.compile()
        nc.finalize()
        _COMPILED[key] = nc
    return _COMPILED[key]


def make_blobs(W, b, question_sequence_i, question_mask_i):
    """Pack per-core constants into the fp16 and fp32 setup blobs."""
    bw = np.zeros((128, BW_F), np.float16)
    wTp = np.zeros((EP, H), np.float16)
    wTp[:E] = W.astype(np.float16).T
    wTp[E] = b.astype(np.float16)          # bias rides the aug row
    for k in range(3):
        bw[:, k * 128 : (k + 1) * 128] = wTp[k * 128 : (k + 1) * 128]
    bq = np.zeros((128, BQ_F), np.float16)
    qTp = np.zeros((EP, QST), np.float16)
    qTp[:E] = question_sequence_i.astype(np.float16).T
    qTp[E] = 1.0                           # ones row pairs with the bias row
    for k in range(3):
        bq[:, k * 512 : (k + 1) * 512] = qTp[k * 128 : (k + 1) * 128]

    b32 = np.zeros((128, BLOB32_F), np.float32)
    b32[:, OFF_B] = b.astype(np.float32)
    mb = np.where(question_mask_i == 0, np.float32(-1e30), np.float32(0.0))
    b32[:, OFF_MB : OFF_MB + 4] = mb.reshape(4, 128).T
    return bw, bq, b32


def make_in_maps(context_sequence, question_sequence, question_mask, W, b):
    in_maps = []
    for i in range(N_CORES):
        xcT = np.zeros((EP, CTX), np.float16)
        xcT[:E] = context_sequence[i].T.astype(np.float16)
        xcT[E] = 1.0                       # ones row pairs with the bias row
        # partition-major tiling: [st, p, k, c] with 3KB contiguous rows
        xc4 = np.ascontiguousarray(
            xcT.reshape(3, 128, N_ST, ST).transpose(2, 1, 0, 3))
        bw, bq, b32 = make_blobs(W, b, question_sequence[i], question_mask[i])
        in_maps.append({"xc4": xc4, "bw": bw, "bq": bq, "b32": b32})
    return in_maps


def assemble_out(res):
    outs = []
    for i in range(N_CORES):
        o4 = res.results[i]["out4"]        # [st, p, ci, h]
        outs.append(o4.transpose(0, 2, 1, 3).reshape(CTX, H))
    return np.stack(outs, axis=0).astype(np.float32)


def kernel(context_sequence, question_sequence, question_mask, W, b):
    from concourse.bass_utils import run_bass_kernel_spmd

    masked = bool(np.any(np.asarray(question_mask) == 0))
    nc = _get_nc(masked=masked)
    in_maps = make_in_maps(
        context_sequence, question_sequence, question_mask, W, b)
    res = run_bass_kernel_spmd(nc, in_maps, core_ids=list(range(N_CORES)))
    return assemble_out(res)


# revision 30
# speedup vs baseline: 1.5269x; 1.5269x over previous
"""Trainium2 Bass kernel for AlignedQuestionEmbeddingLayer.

Computation (per batch element):
    C = relu(Xc @ W.T + b)            # [4096, 128]
    Q = relu(Xq @ W.T + b)            # [512, 128]
    S = C @ Q.T  (+ mask)             # [4096, 512]
    A = softmax(S, axis=-1)
    out = A @ Q                       # [4096, 128]

Sharding: data-parallel over batch, one batch element per NeuronCore (8 cores).

Device-side design notes:
  - The dense layer contracts over E=300 (padded to 384 on host), so both
    matmul operands need E on the partition axis. Inputs are pre-transposed
    and fp16-cast on host: no on-device transposes of the big tensor, half
    the input DMA traffic, and fp16 matmuls run 4x faster than fp32 on the
    TRN2 PE (1 cycle/row vs 4).
  - The bias rides the E-padding: row 300 of xT is all-ones and row 300 of
    the W blob holds b, so both denses produce x@W.T+b straight out of the
    matmul and relu is a single DVE max (fp16/bf16 cast included).
  - One input DMA and one output DMA per 512-row super-tile (the Sync
    sequencer's serial DMA-trigger cost dominates otherwise).
  - Scores are computed transposed ([q partitions, c free]) so the final
    matmul (lhsT=expS_T chunk, rhs=[Q|ones]) produces both the output rows
    and the softmax denominators in a single PSUM accumulation; exp runs as
    two 1024-wide activations with bias=0 when the mask is all ones, and
    falls back to 4 per-chunk activations with the mask folded into the
    per-partition exp bias otherwise.
  - Softmax skips max-subtraction: scores are bounded (~|s|<40) so exp() is
    safe in fp32/bf16.
  - The loop is software-pipelined with lag 1 (final/norm/store of st-1
    between dense and scores of st) so the PE never waits on exp.
  - A burst of dummy matmuls at kernel start warms the PE HAM clock gate
    during the setup-DMA window, so real matmuls run at 2.4 GHz not 1.2.
"""

import sys

import numpy as np

sys.path.insert(0, "/opt/trn_rl_repo")

B, CTX, QST, E, H = 8, 4096, 512, 300, 128
N_CORES = 8
EP = 384            # E padded to 3 chunks of 128; row E carries the bias
ST = 512            # context rows per super-tile
N_ST = CTX // ST    # 8 super-tiles
N_WARM = 7          # dummy matmuls to warm the HAM clock gate

# fp16 blobs: W chunks (with bias row) and XqT chunks (with ones row)
BW_F = EP                 # bw: [:, k*128 : +128], k=0..2
BQ_F = 3 * 512            # bq: [:, k*512 : +512], k=0..2
# fp32 blob: bias column + 4 maskbias columns (only read by masked variant)
OFF_B = 0
OFF_MB = 1
BLOB32_F = 5

_COMPILED = {}


def _build_kernel(n_st=N_ST, masked=False):
    import concourse.bass as bass
    import concourse.tile as tile
    from concourse import bacc, mybir

    f32 = mybir.dt.float32
    f16 = mybir.dt.float16
    bf16 = mybir.dt.bfloat16
    AF = mybir.ActivationFunctionType
    MAX = mybir.AluOpType.max

    nc = bacc.Bacc(
        "TRN2", target_bir_lowering=False, debug=False, num_devices=N_CORES
    )

    xc4 = nc.declare_dram_parameter("xc4", [N_ST, 128, 3, ST], f16, isOutput=False)
    bw_d = nc.declare_dram_parameter("bw", [128, BW_F], f16, isOutput=False)
    bq_d = nc.declare_dram_parameter("bq", [128, BQ_F], f16, isOutput=False)
    b32_d = nc.declare_dram_parameter("b32", [128, BLOB32_F], f32, isOutput=False)
    out_d = nc.declare_dram_parameter("out4", [N_ST, 128, ST // 128, H], f32,
                                      isOutput=True)

    with tile.TileContext(nc) as tc:
        with (
            tc.tile_pool(name="const", bufs=1) as const_pool,
            tc.tile_pool(name="xin", bufs=4) as xin_pool,
            tc.tile_pool(name="ct", bufs=2) as ct_pool,
            tc.tile_pool(name="exps", bufs=2) as exps_pool,
            tc.tile_pool(name="outs", bufs=3) as outs_pool,
            tc.tile_pool(name="pct", bufs=1, space=bass.MemorySpace.PSUM) as pct_pool,
            tc.tile_pool(name="pst", bufs=2, space=bass.MemorySpace.PSUM) as pst_pool,
            tc.tile_pool(name="po", bufs=3, space=bass.MemorySpace.PSUM) as po_pool,
        ):
            # ---- PE warmup: matmuls on an uninitialized tile, results
            # discarded (next dense's start=True clears the bank) ----------
            warm = const_pool.tile([128, ST], f16, tag="warm")
            nc.gpsimd.memset(warm[:], 0.0)
            warm_ps = pct_pool.tile([H, ST], f32, tag="pct")
            for _ in range(N_WARM):
                nc.tensor.matmul(
                    warm_ps[:], warm[:, 0:128], warm[:], start=True, stop=True,
                    skip_group_check=True,
                )

            # ---- one-time setup -------------------------------------------
            bw_sb = const_pool.tile([128, BW_F], f16, tag="bw")
            nc.sync.dma_start(bw_sb[:], bw_d[:])
            bq_sb = const_pool.tile([128, BQ_F], f16, tag="bq")

            def load_bq():
                nc.sync.dma_start(bq_sb[:], bq_d[:])

            if masked:
                setup32 = const_pool.tile([128, BLOB32_F], f32, tag="setup32")
                nc.sync.dma_start(setup32[:], b32_d[:])

            def w_chunk(k):  # [128, 128] fp16 W.T chunk (row E holds b)
                return bw_sb[:, k * 128 : (k + 1) * 128]

            def q_chunk(k):  # [128, 512] fp16 Xq.T chunk (row E all-ones)
                return bq_sb[:, k * 512 : (k + 1) * 512]

            # question dense twice (bias rides the aug row):
            #   QT [h, q] fp16 for the scores lhsT
            #   Q  [q, h] bf16 (as [Q|ones] per chunk) for the final rhs
            qt_sb = const_pool.tile([H, QST], f16, tag="qt")
            qa_sb = []
            for j in range(4):
                qa_tile = const_pool.tile([128, H + 1], bf16, tag=f"qa{j}")
                qa_sb.append(qa_tile)

            def qt_setup():
                psum_q = pct_pool.tile([H, QST], f32, tag="pct")
                for k in range(3):
                    nc.tensor.matmul(
                        psum_q[:], w_chunk(k), q_chunk(k),
                        start=(k == 0), stop=(k == 2),
                    )
                nc.vector.tensor_scalar(qt_sb[:], psum_q[:], 0.0, None, MAX)

            def qa_setup():
                psum_qd = pst_pool.tile([128, 2 * ST], f32, tag="pst")
                for j in range(4):
                    for k in range(3):
                        nc.tensor.matmul(
                            psum_qd[:, j * 128 : (j + 1) * 128],
                            q_chunk(k)[:, j * 128 : (j + 1) * 128], w_chunk(k),
                            start=(k == 0), stop=(k == 2),
                        )
                for j in range(4):
                    nc.vector.tensor_scalar(
                        qa_sb[j][:, 0:H], psum_qd[:, j * 128 : (j + 1) * 128],
                        0.0, None, MAX,
                    )
                    nc.gpsimd.memset(qa_sb[j][:, H : H + 1], 1.0)

            # ---- software-pipelined main loop -----------------------------
            def load_phase(st, split=False):
                xa = xin_pool.tile([128, 3, ST], f16, tag="xa")
                if split:  # per-chunk DMAs so dense k=0 can start earliest
                    for k in range(3):
                        nc.sync.dma_start(xa[:, k, :], xc4[st, :, k, :])
                else:
                    nc.sync.dma_start(xa[:], xc4[st])
                return xa

            def dense_phase(xa):
                psum_ct = pct_pool.tile([H, ST], f32, tag="pct")
                for k in range(3):
                    nc.tensor.matmul(
                        psum_ct[:], w_chunk(k), xa[:, k, :],
                        start=(k == 0), stop=(k == 2),
                    )
                ct_sb = ct_pool.tile([H, ST], f16, tag="ct")
                nc.vector.tensor_scalar(ct_sb[:], psum_ct[:], 0.0, None, MAX)
                return ct_sb

            def scores_exp_phase(ct_sb):
                es = exps_pool.tile([128, 4 * ST], bf16, tag="es")
                for half in range(2):
                    ps = pst_pool.tile([128, 2 * ST], f32, tag="pst")
                    for jj in range(2):
                        j = 2 * half + jj
                        nc.tensor.matmul(
                            ps[:, jj * ST : (jj + 1) * ST],
                            qt_sb[:, j * 128 : (j + 1) * 128], ct_sb[:],
                            start=True, stop=True,
                        )
                    eslice = es[:, half * 2 * ST : (half + 1) * 2 * ST]
                    if masked:
                        for jj in range(2):
                            j = 2 * half + jj
                            nc.scalar.activation(
                                eslice[:, jj * ST : (jj + 1) * ST],
                                ps[:, jj * ST : (jj + 1) * ST], AF.Exp,
                                bias=setup32[:, OFF_MB + j : OFF_MB + j + 1],
                            )
                    else:
                        nc.scalar.activation(eslice, ps[:], AF.Exp)
                return es

            def back_phase(st, es):
                def exp_chunk(ci, j):  # [128 q, 128 c] bf16 lhsT
                    return es[:, j * ST + ci * 128 : j * ST + (ci + 1) * 128]

                o_big = outs_pool.tile([128, ST // 128, H], f32, tag="obig")
                for cp in range(ST // 256):  # ci pairs share one PSUM bank
                    po2 = po_pool.tile([128, 2, H + 1], f32, tag="po")
                    for ch in range(2):
                        ci = 2 * cp + ch
                        for j in range(4):
                            nc.tensor.matmul(
                                po2[:, ch, :], exp_chunk(ci, j), qa_sb[j][:],
                                start=(j == 0), stop=(j == 3),
                            )
                    recip2 = outs_pool.tile([128, 2], f32, tag="recip")
                    sums2 = po2[:, :, H : H + 1].rearrange("p a b -> p (a b)")
                    nc.vector.reciprocal(recip2[:], sums2)
                    for ch in range(2):
                        ci = 2 * cp + ch
                        nc.vector.tensor_scalar_mul(
                            o_big[:, ci, :], po2[:, ch, 0:H],
                            recip2[:, ch : ch + 1])
                nc.sync.dma_start(out_d[st], o_big[:])

            # prologue: kick off the first three x loads, QT dense while
            # they land, dense+scores of st 0 as soon as x0 arrives, and
            # defer the (final-only) Q/qa dense past the first scores
            xas = {0: load_phase(0, split=True)}
            load_bq()
            for _st in range(1, min(3, n_st)):
                xas[_st] = load_phase(_st)
            cts = {0: dense_phase(xas.pop(0))}
            qt_setup()
            qa_setup()
            if n_st > 1:
                cts[1] = dense_phase(xas.pop(1))
            prev_exp = scores_exp_phase(cts[0])
            for st in range(1, n_st + 1):
                if st < n_st:
                    if st + 2 < n_st:
                        xas[st + 2] = load_phase(st + 2)
                    if st + 1 < n_st:
                        cts[st + 1] = dense_phase(xas.pop(st + 1))
                    back_phase(st - 1, prev_exp)
                    prev_exp = scores_exp_phase(cts.pop(st))
                else:
                    back_phase(st - 1, prev_exp)

    return nc


def _get_nc(masked=False):
    key = ("nc", masked)
    if key not in _COMPILED:
        nc = _build_kernel(masked=masked)
        nc.compile()
        nc.finalize()
        _COMPILED[key] = nc
    return _COMPILED[key]


def make_blobs(W, b, question_sequence_i, question_mask_i):
    """Pack per-core constants into the fp16 and fp32 setup blobs."""
    bw = np.zeros((128, BW_F), np.float16)
    wTp = np.zeros((EP, H), np.float16)
    wTp[:E] = W.astype(np.float16).T
    wTp[E] = b.astype(np.float16)          # bias rides the aug row
    for k in range(3):
        bw[:, k * 128 : (k + 1) * 128] = wTp[k * 128 : (k + 1) * 128]
    bq = np.zeros((128, BQ_F), np.float16)
    qTp = np.zeros((EP, QST), np.float16)
    qTp[:E] = question_sequence_i.astype(np.float16).T
    qTp[E] = 1.0                           # ones row pairs with the bias row
    for k in range(3):
        bq[:, k * 512 : (k + 1) * 512] = qTp[k * 128 : (k + 1) * 128]

    b32 = np.zeros((128, BLOB32_F), np.float32)
    b32[:, OFF_B] = b.astype(np.float32)
    mb = np.where(question_mask_i == 0, np.float32(-1e30), np.float32(0.0))
    b32[:, OFF_MB : OFF_MB + 4] = mb.reshape(4, 128).T
    return bw, bq, b32


def make_in_maps(context_sequence, question_sequence, question_mask, W, b):
    in_maps = []
    for i in range(N_CORES):
        xcT = np.zeros((EP, CTX), np.float16)
        xcT[:E] = context_sequence[i].T.astype(np.float16)
        xcT[E] = 1.0                       # ones row pairs with the bias row
        # partition-major tiling: [st, p, k, c] with 3KB contiguous rows
        xc4 = np.ascontiguousarray(
            xcT.reshape(3, 128, N_ST, ST).transpose(2, 1, 0, 3))
        bw, bq, b32 = make_blobs(W, b, question_sequence[i], question_mask[i])
        in_maps.append({"xc4": xc4, "bw": bw, "bq": bq, "b32": b32})
    return in_maps


def assemble_out(res):
    outs = []
    for i in range(N_CORES):
        o4 = res.results[i]["out4"]        # [st, p, ci, h]
        outs.append(o4.transpose(0, 2, 1, 3).reshape(CTX, H))
    return np.stack(outs, axis=0).astype(np.float32)


def kernel(context_sequence, question_sequence, question_mask, W, b):
    from concourse.bass_utils import run_bass_kernel_spmd

    masked = bool(np.any(np.asarray(question_mask) == 0))
    nc = _get_nc(masked=masked)
    in_maps = make_in_maps(
        context_sequence, question_sequence, question_mask, W, b)
    res = run_bass_kernel_spmd(nc, in_maps, core_ids=list(range(N_CORES)))
    return assemble_out(res)
